# revision 11
# baseline (speedup 1.0000x reference)
"""AdaptivePoolAttention Trainium2 kernel (8 NeuronCores, SPMD).

Key algebraic restructure: AdaptiveAvgPool3d over spatial (H,W) commutes with
the qkv linear projection, so we pool x first (mean over H*W) and run the qkv
GEMM on the pooled (B,T,D) tensor. This turns the dominant-GEMM problem into a
memory-bound spatial reduction plus a small attention tail.

Sharding: core c handles batch b=c//2, token half c%2 (32 of 64 tokens).
 - Phase 1: each core pools its 32 tokens (spatial mean via a selector matmul
   on the TensorEngine, accumulated in PSUM).
 - Pairwise AllGather (cores 2c, 2c+1) of the pooled halves -> full (64, 768).
 - Phase 2b (overlaps the collective): q projection + LN + rel-pos bias for the
   local 32 query tokens.
 - Phase 3: k/v projection + LN for all 64 tokens, per-head attention with the
   temporal rel-pos bias folded into the PSUM score accumulation, softmax
   (no max-subtraction: |logits| < 5), A@V, residual, output projection.
Host side only shards/preps inputs and concatenates the 8 disjoint
(32, 768) output row-blocks.
"""

import numpy as np
from contextlib import ExitStack

B, T, NH, HD, D = 4, 64, 12, 64, 768
S = 196            # 14*14 spatial positions
TLOC = 32          # tokens per core
NROWS = TLOC * S   # 6272 rows of x per core
NT = 49            # 128-row tiles per core
G = 7              # tiles per DMA group
KB = D // 128      # 6 contraction tiles of 128
SCALE = HD ** -0.5
LN_EPS = 1e-5
N_CORES = 8

_BUILD_CACHE = {}


def _build_nc():
    import concourse.bass as bass
    import concourse.bacc as bacc
    import concourse.tile as tile
    import concourse.mybir as mybir
    from concourse.masks import make_identity

    f32 = mybir.dt.float32
    f32r = mybir.dt.float32r

    nc = bacc.Bacc(
        "TRN2", target_bir_lowering=False, debug=False, num_devices=N_CORES,
    )

    xloc = nc.declare_dram_parameter("xloc", [NROWS, D], f32r, isOutput=False)
    sel = nc.declare_dram_parameter("sel", [NROWS, TLOC], f32r, isOutput=False)
    wq = nc.declare_dram_parameter("wq", [D, D], f32r, isOutput=False)
    wkv = nc.declare_dram_parameter("wkv", [D, 2 * D], f32r, isOutput=False)
    wp = nc.declare_dram_parameter("wp", [D, D], f32r, isOutput=False)
    rtt = nc.declare_dram_parameter("rtt", [TLOC, HD, T], f32, isOutput=False)
    gq = nc.declare_dram_parameter("gq", [TLOC, D], f32, isOutput=False)
    bq = nc.declare_dram_parameter("bq", [TLOC, D], f32, isOutput=False)
    gkv = nc.declare_dram_parameter("gkv", [T, 2 * D], f32, isOutput=False)
    bkv = nc.declare_dram_parameter("bkv", [T, 2 * D], f32, isOutput=False)
    bproj = nc.declare_dram_parameter("bproj", [TLOC, D], f32, isOutput=False)
    out_ext = nc.declare_dram_parameter("out", [TLOC, D], f32, isOutput=True)

    with ExitStack() as ctx:
        tc = ctx.enter_context(tile.TileContext(nc))
        const = ctx.enter_context(tc.tile_pool(name="const", bufs=1))
        xp = ctx.enter_context(tc.tile_pool(name="xp", bufs=2))
        sb = ctx.enter_context(tc.tile_pool(name="sb", bufs=1))
        pg = ctx.enter_context(tc.tile_pool(name="pg", bufs=2, space="PSUM"))
        pt = ctx.enter_context(tc.tile_pool(name="pt", bufs=2, space="PSUM"))
        dram = ctx.enter_context(tc.tile_pool(name="dram", bufs=1, space="DRAM"))

        ident = const.tile([128, 128], f32, tag="ident")
        make_identity(nc, ident)
        eps_sb = const.tile([128, 1], f32, tag="eps")
        nc.vector.memset(eps_sb, LN_EPS)
        zero_sb = const.tile([128, 1], f32, tag="zero")
        nc.vector.memset(zero_sb, 0.0)

        # ---- constants / weights to SBUF ----
        sel_sb = const.tile([128, NT, TLOC], f32r, tag="sel")
        nc.sync.dma_start(out=sel_sb, in_=sel.ap().rearrange("(i p) j -> p i j", p=128))
        wq_sb = const.tile([128, KB, D], f32r, tag="wq")
        nc.scalar.dma_start(out=wq_sb, in_=wq.ap().rearrange("(k p) e -> p k e", p=128))
        wkv_sb = const.tile([128, KB, 2 * D], f32r, tag="wkv")
        nc.scalar.dma_start(out=wkv_sb, in_=wkv.ap().rearrange("(k p) e -> p k e", p=128))
        wp_sb = const.tile([128, KB, D], f32r, tag="wp")
        nc.scalar.dma_start(out=wp_sb, in_=wp.ap().rearrange("(k p) e -> p k e", p=128))
        rtt_sb = const.tile([HD, TLOC, T], f32, tag="rtt")
        nc.scalar.dma_start(out=rtt_sb, in_=rtt.ap().rearrange("t d s -> d t s"))
        gq_sb = const.tile([TLOC, D], f32, tag="gq")
        nc.gpsimd.dma_start(out=gq_sb, in_=gq.ap())
        bq_sb = const.tile([TLOC, D], f32, tag="bq")
        nc.gpsimd.dma_start(out=bq_sb, in_=bq.ap())
        gkv_sb = const.tile([T, 2 * D], f32, tag="gkv")
        nc.gpsimd.dma_start(out=gkv_sb, in_=gkv.ap())
        bkv_sb = const.tile([T, 2 * D], f32, tag="bkv")
        nc.gpsimd.dma_start(out=bkv_sb, in_=bkv.ap())
        bproj_sb = const.tile([TLOC, D], f32, tag="bproj")
        nc.gpsimd.dma_start(out=bproj_sb, in_=bproj.ap())

        # ---- phase 1: spatial pooling (selector matmul, PSUM accumulate) ----
        x_r = xloc.ap().rearrange("(g i p) d -> g p i d", i=G, p=128)
        m_psum = pg.tile([TLOC, D], f32, tag="g")
        for g in range(G):
            xt = xp.tile([128, G, D], f32r, tag="x")
            nc.sync.dma_start(out=xt, in_=x_r[g])
            for i in range(G):
                ti = g * G + i
                for c0, cw in ((0, 512), (512, 256)):
                    nc.tensor.matmul(
                        m_psum[:, c0:c0 + cw],
                        sel_sb[:, ti, :],
                        xt[:, i, c0:c0 + cw],
                        start=(ti == 0),
                        stop=(ti == NT - 1),
                    )
        m_sb = sb.tile([TLOC, D], f32, tag="m")
        nc.vector.tensor_copy(out=m_sb, in_=m_psum)

        # ---- pairwise AllGather of pooled halves ----
        ag_in = dram.tile([TLOC, D], f32, tag="agi")
        ag_out = dram.tile([T, D], f32, tag="ago")
        nc.gpsimd.dma_start(out=ag_in, in_=m_sb)
        nc.gpsimd.collective_compute(
            "AllGather",
            mybir.AluOpType.bypass,
            replica_groups=[[0, 1], [2, 3], [4, 5], [6, 7]],
            ins=[ag_in.opt()],
            outs=[ag_out.opt()],
        )
        mf_sb = sb.tile([T, D], f32, tag="mf")
        nc.gpsimd.dma_start(out=mf_sb, in_=ag_out)

        def bcast_free(ap2d, inner):
            # (P, F) AP -> (P, F, inner) AP with stride-0 innermost broadcast
            return bass.AP(
                tensor=ap2d.tensor,
                offset=ap2d.offset,
                ap=[*ap2d.ap, [0, inner]],
            )

        def layer_norm(src_psum, n_part, n_groups, g_tile, b_tile, out_tile, nm):
            # src (n_part, n_groups*64): per-64-group LN, batched DVE ops
            src3 = src_psum.rearrange("p (g d) -> p g d", g=n_groups)
            mean = sb.tile([n_part, n_groups], f32, tag=f"{nm}_mean")
            nc.vector.reduce_sum(out=mean, in_=src3, axis=mybir.AxisListType.X)
            nc.vector.tensor_scalar_mul(out=mean, in0=mean, scalar1=1.0 / HD)
            xc = sb.tile([n_part, n_groups, HD], f32, tag=f"{nm}_xc")
            nc.vector.tensor_tensor(
                out=xc, in0=src3, in1=bcast_free(mean[:], HD),
                op=mybir.AluOpType.subtract,
            )
            sq = sb.tile([n_part, n_groups, HD], f32, tag=f"{nm}_sq")
            nc.vector.tensor_mul(out=sq, in0=xc, in1=xc)
            var = sb.tile([n_part, n_groups], f32, tag=f"{nm}_var")
            nc.vector.reduce_sum(out=var, in_=sq, axis=mybir.AxisListType.X)
            # std = sqrt(var/HD + eps); rstd = 1/std
            nc.scalar.activation(
                out=var, in_=var, func=mybir.ActivationFunctionType.Sqrt,
                bias=eps_sb[:n_part], scale=1.0 / HD,
            )
            nc.vector.reciprocal(out=var, in_=var)
            nc.vector.tensor_tensor(
                out=xc, in0=xc, in1=bcast_free(var[:], HD),
                op=mybir.AluOpType.mult,
            )
            xcf = xc.rearrange("p g d -> p (g d)")
            nc.vector.tensor_mul(out=xcf, in0=xcf, in1=g_tile)
            nc.vector.tensor_add(out=out_tile, in0=xcf, in1=b_tile)

        # ---- phase 2b: q path (local tokens; independent of collective) ----
        mT_psum = pt.tile([128, KB, TLOC], f32, tag="t")
        for k in range(KB):
            nc.tensor.matmul(
                mT_psum[:, k, :], m_sb[:, k * 128:(k + 1) * 128],
                ident[:TLOC, :TLOC], is_transpose=True,
            )
        mT_sb = sb.tile([128, KB, TLOC], f32r, tag="mT")
        nc.vector.tensor_copy(out=mT_sb, in_=mT_psum)

        q_psum = pg.tile([TLOC, D], f32, tag="g")
        for k in range(KB):
            for c0, cw in ((0, 512), (512, 256)):
                nc.tensor.matmul(
                    q_psum[:, c0:c0 + cw],
                    mT_sb[:, k, :],
                    wq_sb[:, k, c0:c0 + cw],
                    start=(k == 0), stop=(k == KB - 1),
                )
        ln_q = sb.tile([TLOC, D], f32, tag="lnq")
        layer_norm(q_psum, TLOC, NH, gq_sb, bq_sb, ln_q, "q")

        # q^T in per-head layout (64 d, NH heads, 32 t)
        qbT_psum = pt.tile([HD, NH, TLOC], f32, tag="t")
        for h in range(NH):
            nc.tensor.matmul(
                qbT_psum[:, h, :], ln_q[:, h * HD:(h + 1) * HD],
                ident[:TLOC, :TLOC], is_transpose=True,
            )
        qbT_sb = sb.tile([HD, NH, TLOC], f32, tag="qbT")
        nc.vector.tensor_copy(out=qbT_sb, in_=qbT_psum)

        # rel-pos bias: bias[s, t, h] = sum_d rtt[t][d, s] * q[t, h, d]
        bias_psum = pt.tile([T, TLOC, NH], f32, tag="t")
        for t in range(TLOC):
            nc.tensor.matmul(
                bias_psum[:, t, :],
                rtt_sb[:, t, :],
                qbT_sb[:, :, t],
                start=True, stop=True,
            )
        bias_sb = sb.tile([T, TLOC, NH], f32, tag="bias")
        nc.vector.tensor_copy(out=bias_sb, in_=bias_psum)

        # ---- phase 3: kv path on gathered tokens ----
        mfT_psum = pt.tile([128, KB, T], f32, tag="t")
        for k in range(KB):
            nc.tensor.matmul(
                mfT_psum[:, k, :], mf_sb[:, k * 128:(k + 1) * 128],
                ident[:T, :T], is_transpose=True,
            )
        mfT_sb = sb.tile([128, KB, T], f32r, tag="mfT")
        nc.vector.tensor_copy(out=mfT_sb, in_=mfT_psum)

        k_psum = pg.tile([T, D], f32, tag="g")
        v_psum = pg.tile([T, D], f32, tag="g")
        for k in range(KB):
            for c0, cw in ((0, 512), (512, 256)):
                nc.tensor.matmul(
                    k_psum[:, c0:c0 + cw],
                    mfT_sb[:, k, :],
                    wkv_sb[:, k, c0:c0 + cw],
                    start=(k == 0), stop=(k == KB - 1),
                )
            for c0, cw in ((0, 512), (512, 256)):
                nc.tensor.matmul(
                    v_psum[:, c0:c0 + cw],
                    mfT_sb[:, k, :],
                    wkv_sb[:, k, D + c0:D + c0 + cw],
                    start=(k == 0), stop=(k == KB - 1),
                )
        ln_k = sb.tile([T, D], f32, tag="lnk")
        layer_norm(k_psum, T, NH, gkv_sb[:, :D], bkv_sb[:, :D], ln_k, "k")
        ln_v = sb.tile([T, D], f32, tag="lnv")
        layer_norm(v_psum, T, NH, gkv_sb[:, D:], bkv_sb[:, D:], ln_v, "v")

        # k^T per head: (64 d, NH, 64 s), all heads based at partition 0
        kT_psum = pt.tile([HD, NH, T], f32, tag="t")
        for h in range(NH):
            nc.tensor.matmul(
                kT_psum[:, h, :], ln_k[:, h * HD:(h + 1) * HD],
                ident[:T, :T], is_transpose=True,
            )
        kT_sb = sb.tile([HD, NH, T], f32, tag="kT")
        nc.vector.tensor_copy(out=kT_sb, in_=kT_psum)

        # scores + bias accumulation, per head
        s_psum = pg.tile([TLOC, NH, T], f32, tag="g")
        for h in range(NH):
            nc.tensor.matmul(
                s_psum[:, h, :], qbT_sb[:, h, :], kT_sb[:, h, :],
                start=True, stop=False,
            )
            nc.tensor.matmul(
                s_psum[:, h, :], bias_sb[:, :, h], ident[:T, :T],
                is_transpose=True, start=False, stop=True,
            )

        # softmax (no max subtraction; |logits| < 5)
        p_sb = sb.tile([TLOC, NH, T], f32, tag="p")
        nc.scalar.activation(
            out=p_sb.rearrange("p h s -> p (h s)"),
            in_=s_psum.rearrange("p h s -> p (h s)"),
            func=mybir.ActivationFunctionType.Exp,
            bias=zero_sb[:TLOC], scale=SCALE,
        )
        rsum = sb.tile([TLOC, NH], f32, tag="rsum")
        nc.vector.reduce_sum(out=rsum, in_=p_sb, axis=mybir.AxisListType.X)
        nc.vector.reciprocal(out=rsum, in_=rsum)

        # P^T per head
        pT_psum = pt.tile([T, NH, TLOC], f32, tag="t")
        for h in range(NH):
            nc.tensor.matmul(
                pT_psum[:, h, :], p_sb[:, h, :],
                ident[:TLOC, :TLOC], is_transpose=True,
            )
        pT_sb = sb.tile([T, NH, TLOC], f32, tag="pT")
        nc.vector.tensor_copy(out=pT_sb, in_=pT_psum)

        # A@V per head
        o_psum = pg.tile([TLOC, NH, HD], f32, tag="g")
        for h in range(NH):
            nc.tensor.matmul(
                o_psum[:, h, :], pT_sb[:, h, :],
                ln_v[:, h * HD:(h + 1) * HD],
                start=True, stop=True,
            )
        # o = o * (1/sum) + ln_q (residual)
        o_sb = sb.tile([TLOC, D], f32, tag="o")
        o3 = o_sb.rearrange("p (h d) -> p h d", h=NH)
        for h in range(NH):
            nc.vector.scalar_tensor_tensor(
                out=o3[:, h, :], in0=o_psum[:, h, :],
                scalar=rsum[:, h:h + 1], in1=ln_q[:, h * HD:(h + 1) * HD],
                op0=mybir.AluOpType.mult, op1=mybir.AluOpType.add,
            )

        # o^T then output projection
        oT_psum = pt.tile([128, KB, TLOC], f32, tag="t")
        for k in range(KB):
            nc.tensor.matmul(
                oT_psum[:, k, :], o_sb[:, k * 128:(k + 1) * 128],
                ident[:TLOC, :TLOC], is_transpose=True,
            )
        oT_sb = sb.tile([128, KB, TLOC], f32r, tag="oT")
        nc.vector.tensor_copy(out=oT_sb, in_=oT_psum)

        proj_psum = pg.tile([TLOC, D], f32, tag="g")
        for k in range(KB):
            for c0, cw in ((0, 512), (512, 256)):
                nc.tensor.matmul(
                    proj_psum[:, c0:c0 + cw],
                    oT_sb[:, k, :],
                    wp_sb[:, k, c0:c0 + cw],
                    start=(k == 0), stop=(k == KB - 1),
                )
        out_sb = sb.tile([TLOC, D], f32, tag="outsb")
        nc.vector.tensor_add(out=out_sb, in0=proj_psum, in1=bproj_sb)
        nc.gpsimd.dma_start(out=out_ext.ap(), in_=out_sb)

    nc.compile()
    return nc


def _host_prep(x, W_qkv, g_q, b_q, g_k, b_k, g_v, b_v, W_proj, b_proj, rel_pos_t):
    x = np.ascontiguousarray(np.asarray(x, np.float32))
    W_qkv = np.asarray(W_qkv, np.float32)
    W_proj = np.ascontiguousarray(np.asarray(W_proj, np.float32))
    rel_pos_t = np.asarray(rel_pos_t, np.float32)

    sel = np.zeros((NROWS, TLOC), np.float32)
    sel[np.arange(NROWS), np.arange(NROWS) // S] = 1.0 / S
    dist = np.arange(T)[:, None] - np.arange(T)[None, :] + (T - 1)
    Rt_eff = rel_pos_t[dist] / SCALE                       # (T, T, HD)
    wq = np.ascontiguousarray(W_qkv[:, :D])
    wkv = np.ascontiguousarray(W_qkv[:, D:])
    gq_b = np.ascontiguousarray(np.broadcast_to(np.tile(np.asarray(g_q, np.float32), NH), (TLOC, D)))
    bq_b = np.ascontiguousarray(np.broadcast_to(np.tile(np.asarray(b_q, np.float32), NH), (TLOC, D)))
    gkv_row = np.concatenate([np.tile(np.asarray(g_k, np.float32), NH),
                              np.tile(np.asarray(g_v, np.float32), NH)])
    bkv_row = np.concatenate([np.tile(np.asarray(b_k, np.float32), NH),
                              np.tile(np.asarray(b_v, np.float32), NH)])
    gkv_b = np.ascontiguousarray(np.broadcast_to(gkv_row, (T, 2 * D)))
    bkv_b = np.ascontiguousarray(np.broadcast_to(bkv_row, (T, 2 * D)))
    bproj_b = np.ascontiguousarray(np.broadcast_to(np.asarray(b_proj, np.float32), (TLOC, D)))

    in_maps = []
    for c in range(N_CORES):
        b = c // 2
        t0 = (c % 2) * TLOC
        rtt_c = np.ascontiguousarray(
            Rt_eff[t0:t0 + TLOC].transpose(0, 2, 1))   # (32, HD, T)
        in_maps.append({
            "xloc": np.ascontiguousarray(x[b, t0:t0 + TLOC].reshape(NROWS, D)),
            "sel": sel,
            "wq": wq,
            "wkv": wkv,
            "wp": W_proj,
            "rtt": rtt_c,
            "gq": gq_b, "bq": bq_b,
            "gkv": gkv_b, "bkv": bkv_b,
            "bproj": bproj_b,
        })
    return in_maps


def _get_nc():
    if "nc" not in _BUILD_CACHE:
        _BUILD_CACHE["nc"] = _build_nc()
    return _BUILD_CACHE["nc"]


def run_on_device(in_maps, **kw):
    from concourse.bass_utils import run_bass_kernel_spmd
    nc = _get_nc()
    return run_bass_kernel_spmd(nc, in_maps, list(range(N_CORES)), **kw)


def kernel(**inputs):
    in_maps = _host_prep(**inputs)
    res = run_on_device(in_maps)
    out = np.zeros((B, T, D), np.float32)
    for c in range(N_CORES):
        b = c // 2
        t0 = (c % 2) * TLOC
        out[b, t0:t0 + TLOC] = res.results[c]["out"]
    return out


# revision 13
# speedup vs baseline: 1.4105x; 1.4105x over previous
"""AdaptivePoolAttention Trainium2 kernel (8 NeuronCores, SPMD).

Key algebraic restructure: AdaptiveAvgPool3d over spatial (H,W) commutes with
the qkv linear projection, so we pool x first (mean over H*W) and run the qkv
GEMM on the pooled (B,T,D) tensor. This turns the dominant-GEMM problem into a
memory-bound spatial reduction plus a small attention tail.

Sharding: core c handles batch b=c//2, token half c%2 (32 of 64 tokens).
 - Phase 1: each core pools its 32 tokens (spatial mean via a bf16 selector
   matmul on the TensorEngine, accumulated in PSUM).
 - Pairwise AllGather (cores 2c, 2c+1) of the pooled halves -> full (64, 768).
 - Phase 2b (overlaps the collective): q projection + LN for the local 32
   query tokens, plus the temporal rel-pos bias. The bias
   bias[t,s,h] = sum_d q[t,h,d] * rel_pos_t[t-s+63, d] is computed as one
   GEMM G = q @ R^T over the 127 distinct distances, then gathered into
   (t, h, s) layout with a single affine-strided DMA through DRAM
   (address = t*1535 + h*128 + s + 63 after a per-core column flip of R).
 - Phase 3: k/v projection + LN for all 64 tokens, per-head attention,
   softmax (no max-subtraction: |logits| < 5), A@V, residual, out projection.
Host side shards/preps inputs (incl. fp32->bf16 for the big operands) and
concatenates the 8 disjoint (32, 768) output row-blocks.
"""

import numpy as np
from contextlib import ExitStack

B, T, NH, HD, D = 4, 64, 12, 64, 768
S = 196            # 14*14 spatial positions
TLOC = 32          # tokens per core
NROWS = TLOC * S   # 6272 rows of x per core
NT = 49            # 128-row tiles per core
G = 7              # tiles per DMA group
KB = D // 128      # 6 contraction tiles of 128
NDIST = 2 * T - 1  # 127 distinct temporal distances
SCALE = HD ** -0.5
LN_EPS = 1e-5
N_CORES = 8

_BUILD_CACHE = {}


def _build_nc():
    import concourse.bass as bass
    import concourse.bacc as bacc
    import concourse.tile as tile
    import concourse.mybir as mybir
    from concourse.masks import make_identity
    from concourse.tile_rust import add_dep_helper

    f32 = mybir.dt.float32
    bf16 = mybir.dt.bfloat16

    nc = bacc.Bacc(
        "TRN2", target_bir_lowering=False, debug=False, num_devices=N_CORES,
    )

    xloc = nc.declare_dram_parameter("xloc", [NROWS, D], bf16, isOutput=False)
    sel = nc.declare_dram_parameter("sel", [NROWS, TLOC], bf16, isOutput=False)
    wq = nc.declare_dram_parameter("wq", [D, D], bf16, isOutput=False)
    wkv = nc.declare_dram_parameter("wkv", [D, 2 * D], bf16, isOutput=False)
    wp = nc.declare_dram_parameter("wp", [D, D], bf16, isOutput=False)
    rptt = nc.declare_dram_parameter("rptt", [HD, 128], bf16, isOutput=False)
    gq = nc.declare_dram_parameter("gq", [TLOC, D], f32, isOutput=False)
    bq = nc.declare_dram_parameter("bq", [TLOC, D], f32, isOutput=False)
    gkv = nc.declare_dram_parameter("gkv", [T, 2 * D], f32, isOutput=False)
    bkv = nc.declare_dram_parameter("bkv", [T, 2 * D], f32, isOutput=False)
    bproj = nc.declare_dram_parameter("bproj", [TLOC, D], f32, isOutput=False)
    out_ext = nc.declare_dram_parameter("out", [TLOC, D], f32, isOutput=True)

    with ExitStack() as ctx:
        tc = ctx.enter_context(tile.TileContext(nc))
        const = ctx.enter_context(tc.tile_pool(name="const", bufs=1))
        xp = ctx.enter_context(tc.tile_pool(name="xp", bufs=2))
        sb = ctx.enter_context(tc.tile_pool(name="sb", bufs=1))
        pg = ctx.enter_context(tc.tile_pool(name="pg", bufs=2, space="PSUM"))
        pt = ctx.enter_context(tc.tile_pool(name="pt", bufs=2, space="PSUM"))
        dram = ctx.enter_context(tc.tile_pool(name="dram", bufs=1, space="DRAM"))

        ident = const.tile([128, 128], bf16, tag="ident")
        make_identity(nc, ident)
        eps_sb = const.tile([128, 1], f32, tag="eps")
        nc.vector.memset(eps_sb, LN_EPS)
        zero_sb = const.tile([128, 1], f32, tag="zero")
        nc.vector.memset(zero_sb, 0.0)

        # ---- phase 1 inputs first on the sync ring: sel, then x groups ----
        sel_sb = const.tile([128, NT, TLOC], bf16, tag="sel")
        nc.sync.dma_start(out=sel_sb, in_=sel.ap().rearrange("(i p) j -> p i j", p=128))

        x_r = xloc.ap().rearrange("(g i p) d -> g p i d", i=G, p=128)
        m_psum = pg.tile([TLOC, D], f32, tag="g")
        x_dmas = []
        for g in range(G):
            xt = xp.tile([128, G, D], bf16, tag="x")
            x_dmas.append(nc.sync.dma_start(out=xt, in_=x_r[g]))
            for i in range(G):
                ti = g * G + i
                for c0, cw in ((0, 512), (512, 256)):
                    nc.tensor.matmul(
                        m_psum[:, c0:c0 + cw],
                        sel_sb[:, ti, :],
                        xt[:, i, c0:c0 + cw],
                        start=(ti == 0),
                        stop=(ti == NT - 1),
                    )
        m_sb = sb.tile([TLOC, D], bf16, tag="m")
        nc.vector.tensor_copy(out=m_sb, in_=m_psum)

        # ---- weights on the sync ring, ordered after the x stream ----
        wq_sb = const.tile([128, KB, D], bf16, tag="wq")
        wd = nc.sync.dma_start(out=wq_sb, in_=wq.ap().rearrange("(k p) e -> p k e", p=128))
        add_dep_helper(wd.ins, x_dmas[-1].ins, sync=False, reason="x before weights")
        wkv_sb = const.tile([128, KB, 2 * D], bf16, tag="wkv")
        wd = nc.sync.dma_start(out=wkv_sb, in_=wkv.ap().rearrange("(k p) e -> p k e", p=128))
        add_dep_helper(wd.ins, x_dmas[-1].ins, sync=False, reason="x before weights")
        wp_sb = const.tile([128, KB, D], bf16, tag="wp")
        wd = nc.sync.dma_start(out=wp_sb, in_=wp.ap().rearrange("(k p) e -> p k e", p=128))
        add_dep_helper(wd.ins, x_dmas[-1].ins, sync=False, reason="x before weights")

        # small constants on gpsimd (SWDGE), any time
        rptt_sb = const.tile([HD, 128], bf16, tag="rptt")
        nc.gpsimd.dma_start(out=rptt_sb, in_=rptt.ap())
        gq_sb = const.tile([TLOC, D], f32, tag="gq")
        nc.gpsimd.dma_start(out=gq_sb, in_=gq.ap())
        bq_sb = const.tile([TLOC, D], f32, tag="bq")
        nc.gpsimd.dma_start(out=bq_sb, in_=bq.ap())
        gkv_sb = const.tile([T, 2 * D], f32, tag="gkv")
        nc.gpsimd.dma_start(out=gkv_sb, in_=gkv.ap())
        bkv_sb = const.tile([T, 2 * D], f32, tag="bkv")
        nc.gpsimd.dma_start(out=bkv_sb, in_=bkv.ap())
        bproj_sb = const.tile([TLOC, D], f32, tag="bproj")
        nc.gpsimd.dma_start(out=bproj_sb, in_=bproj.ap())

        # ---- pairwise AllGather of pooled halves (bf16) ----
        ag_in = dram.tile([TLOC, D], bf16, tag="agi")
        ag_out = dram.tile([T, D], bf16, tag="ago")
        nc.gpsimd.dma_start(out=ag_in, in_=m_sb)
        nc.gpsimd.collective_compute(
            "AllGather",
            mybir.AluOpType.bypass,
            replica_groups=[[0, 1], [2, 3], [4, 5], [6, 7]],
            ins=[ag_in.opt()],
            outs=[ag_out.opt()],
        )
        mf_sb = sb.tile([T, D], bf16, tag="mf")
        nc.sync.dma_start(out=mf_sb, in_=ag_out)

        def bcast_free(ap2d, inner):
            # (P, F) AP -> (P, F, inner) AP with stride-0 innermost broadcast
            return bass.AP(
                tensor=ap2d.tensor,
                offset=ap2d.offset,
                ap=[*ap2d.ap, [0, inner]],
            )

        def layer_norm(src_psum, n_part, n_groups, g_tile, b_tile, out_tile, nm):
            # src (n_part, n_groups*64): per-64-group LN, batched DVE ops
            src3 = src_psum.rearrange("p (g d) -> p g d", g=n_groups)
            mean = sb.tile([n_part, n_groups], f32, tag=f"{nm}_mean")
            nc.vector.reduce_sum(out=mean, in_=src3, axis=mybir.AxisListType.X)
            nc.vector.tensor_scalar_mul(out=mean, in0=mean, scalar1=1.0 / HD)
            xc = sb.tile([n_part, n_groups, HD], f32, tag=f"{nm}_xc")
            nc.vector.tensor_tensor(
                out=xc, in0=src3, in1=bcast_free(mean[:], HD),
                op=mybir.AluOpType.subtract,
            )
            sq = sb.tile([n_part, n_groups, HD], f32, tag=f"{nm}_sq")
            nc.vector.tensor_mul(out=sq, in0=xc, in1=xc)
            var = sb.tile([n_part, n_groups], f32, tag=f"{nm}_var")
            nc.vector.reduce_sum(out=var, in_=sq, axis=mybir.AxisListType.X)
            # std = sqrt(var/HD + eps); rstd = 1/std
            nc.scalar.activation(
                out=var, in_=var, func=mybir.ActivationFunctionType.Sqrt,
                bias=eps_sb[:n_part], scale=1.0 / HD,
            )
            nc.vector.reciprocal(out=var, in_=var)
            nc.vector.tensor_tensor(
                out=xc, in0=xc, in1=bcast_free(var[:], HD),
                op=mybir.AluOpType.mult,
            )
            xcf = xc.rearrange("p g d -> p (g d)")
            nc.vector.tensor_mul(out=xcf, in0=xcf, in1=g_tile)
            nc.vector.tensor_add(out=out_tile, in0=xcf, in1=b_tile)

        # ---- phase 2b: q path (local tokens; overlaps the collective) ----
        mT_psum = pt.tile([128, KB, TLOC], bf16, tag="t")
        for k in range(KB):
            nc.tensor.matmul(
                mT_psum[:, k, :], m_sb[:, k * 128:(k + 1) * 128],
                ident[:TLOC, :TLOC], is_transpose=True,
            )
        mT_sb = sb.tile([128, KB, TLOC], bf16, tag="mT")
        nc.vector.tensor_copy(out=mT_sb, in_=mT_psum)

        q_psum = pg.tile([TLOC, D], f32, tag="g")
        for k in range(KB):
            for c0, cw in ((0, 512), (512, 256)):
                nc.tensor.matmul(
                    q_psum[:, c0:c0 + cw],
                    mT_sb[:, k, :],
                    wq_sb[:, k, c0:c0 + cw],
                    start=(k == 0), stop=(k == KB - 1),
                )
        ln_q = sb.tile([TLOC, D], bf16, tag="lnq")
        layer_norm(q_psum, TLOC, NH, gq_sb, bq_sb, ln_q, "q")

        # q^T in per-head layout (64 d, NH heads, 32 t)
        qbT_psum = pt.tile([HD, NH, TLOC], bf16, tag="t")
        for h in range(NH):
            nc.tensor.matmul(
                qbT_psum[:, h, :], ln_q[:, h * HD:(h + 1) * HD],
                ident[:TLOC, :TLOC], is_transpose=True,
            )
        qbT_sb = sb.tile([HD, NH, TLOC], bf16, tag="qbT")
        nc.vector.tensor_copy(out=qbT_sb, in_=qbT_psum)

        # rel-pos bias: G[t, h, j] = sum_d q[t,h,d] * Rflip[d, j], then an
        # affine gather through DRAM turns G into bias[t, h, s] (j = 63-t+s).
        g_psum = pg.tile([TLOC, NH, 128], f32, tag="g")
        for h in range(NH):
            nc.tensor.matmul(
                g_psum[:, h, :], qbT_sb[:, h, :], rptt_sb,
                start=True, stop=True,
            )
        g_sb = sb.tile([TLOC, NH, 128], f32, tag="gsb")
        nc.vector.tensor_copy(out=g_sb, in_=g_psum)
        g_dram = dram.tile([TLOC, NH, 128], f32, tag="gd")
        nc.sync.dma_start(out=g_dram, in_=g_sb)
        bias_sb = sb.tile([TLOC, NH, T], f32, tag="bias")
        gather_ap = bass.AP(
            tensor=g_dram.tensor,
            offset=g_dram.offset + 63,
            ap=[[NH * 128 - 1, TLOC], [128, NH], [1, T]],
        )
        nc.sync.dma_start(out=bias_sb, in_=gather_ap)

        # ---- phase 3: kv path on gathered tokens ----
        mfT_psum = pt.tile([128, KB, T], bf16, tag="t")
        for k in range(KB):
            nc.tensor.matmul(
                mfT_psum[:, k, :], mf_sb[:, k * 128:(k + 1) * 128],
                ident[:T, :T], is_transpose=True,
            )
        mfT_sb = sb.tile([128, KB, T], bf16, tag="mfT")
        nc.vector.tensor_copy(out=mfT_sb, in_=mfT_psum)

        k_psum = pg.tile([T, D], f32, tag="g")
        v_psum = pg.tile([T, D], f32, tag="g")
        for k in range(KB):
            for c0, cw in ((0, 512), (512, 256)):
                nc.tensor.matmul(
                    k_psum[:, c0:c0 + cw],
                    mfT_sb[:, k, :],
                    wkv_sb[:, k, c0:c0 + cw],
                    start=(k == 0), stop=(k == KB - 1),
                )
            for c0, cw in ((0, 512), (512, 256)):
                nc.tensor.matmul(
                    v_psum[:, c0:c0 + cw],
                    mfT_sb[:, k, :],
                    wkv_sb[:, k, D + c0:D + c0 + cw],
                    start=(k == 0), stop=(k == KB - 1),
                )
        ln_k = sb.tile([T, D], bf16, tag="lnk")
        layer_norm(k_psum, T, NH, gkv_sb[:, :D], bkv_sb[:, :D], ln_k, "k")
        ln_v = sb.tile([T, D], bf16, tag="lnv")
        layer_norm(v_psum, T, NH, gkv_sb[:, D:], bkv_sb[:, D:], ln_v, "v")

        # k^T per head: (64 d, NH, 64 s)
        kT_psum = pt.tile([HD, NH, T], bf16, tag="t")
        for h in range(NH):
            nc.tensor.matmul(
                kT_psum[:, h, :], ln_k[:, h * HD:(h + 1) * HD],
                ident[:T, :T], is_transpose=True,
            )
        kT_sb = sb.tile([HD, NH, T], bf16, tag="kT")
        nc.vector.tensor_copy(out=kT_sb, in_=kT_psum)

        # scores = q @ k^T, then add bias, then exp (scaled)
        s_psum = pg.tile([TLOC, NH, T], f32, tag="g")
        for h in range(NH):
            nc.tensor.matmul(
                s_psum[:, h, :], qbT_sb[:, h, :], kT_sb[:, h, :],
                start=True, stop=True,
            )
        s_sb = sb.tile([TLOC, NH, T], f32, tag="ssb")
        nc.vector.tensor_add(out=s_sb, in0=s_psum, in1=bias_sb)
        p_sb = sb.tile([TLOC, NH, T], bf16, tag="p")
        nc.scalar.activation(
            out=p_sb.rearrange("p h s -> p (h s)"),
            in_=s_sb.rearrange("p h s -> p (h s)"),
            func=mybir.ActivationFunctionType.Exp,
            bias=zero_sb[:TLOC], scale=SCALE,
        )
        rsum = sb.tile([TLOC, NH], f32, tag="rsum")
        nc.vector.reduce_sum(out=rsum, in_=p_sb, axis=mybir.AxisListType.X)
        nc.vector.reciprocal(out=rsum, in_=rsum)

        # P^T per head
        pT_psum = pt.tile([T, NH, TLOC], bf16, tag="t")
        for h in range(NH):
            nc.tensor.matmul(
                pT_psum[:, h, :], p_sb[:, h, :],
                ident[:TLOC, :TLOC], is_transpose=True,
            )
        pT_sb = sb.tile([T, NH, TLOC], bf16, tag="pT")
        nc.vector.tensor_copy(out=pT_sb, in_=pT_psum)

        # A@V per head
        o_psum = pg.tile([TLOC, NH, HD], f32, tag="g")
        for h in range(NH):
            nc.tensor.matmul(
                o_psum[:, h, :], pT_sb[:, h, :],
                ln_v[:, h * HD:(h + 1) * HD],
                start=True, stop=True,
            )
        # o = o * (1/sum) + ln_q (residual)
        o_nrm = sb.tile([TLOC, NH, HD], bf16, tag="onrm")
        for h in range(NH):
            nc.vector.tensor_scalar_mul(
                out=o_nrm[:, h, :], in0=o_psum[:, h, :], scalar1=rsum[:, h:h + 1],
            )
        o_sb = sb.tile([TLOC, D], bf16, tag="o")
        nc.vector.tensor_add(
            out=o_sb, in0=o_nrm.rearrange("p h d -> p (h d)"), in1=ln_q,
        )

        # o^T then output projection
        oT_psum = pt.tile([128, KB, TLOC], bf16, tag="t")
        for k in range(KB):
            nc.tensor.matmul(
                oT_psum[:, k, :], o_sb[:, k * 128:(k + 1) * 128],
                ident[:TLOC, :TLOC], is_transpose=True,
            )
        oT_sb = sb.tile([128, KB, TLOC], bf16, tag="oT")
        nc.vector.tensor_copy(out=oT_sb, in_=oT_psum)

        proj_psum = pg.tile([TLOC, D], f32, tag="g")
        for k in range(KB):
            for c0, cw in ((0, 512), (512, 256)):
                nc.tensor.matmul(
                    proj_psum[:, c0:c0 + cw],
                    oT_sb[:, k, :],
                    wp_sb[:, k, c0:c0 + cw],
                    start=(k == 0), stop=(k == KB - 1),
                )
        out_sb = sb.tile([TLOC, D], f32, tag="outsb")
        nc.vector.tensor_add(out=out_sb, in0=proj_psum, in1=bproj_sb)
        nc.gpsimd.dma_start(out=out_ext.ap(), in_=out_sb)

    nc.compile()
    return nc


def _host_prep(x, W_qkv, g_q, b_q, g_k, b_k, g_v, b_v, W_proj, b_proj, rel_pos_t):
    import ml_dtypes
    bf = ml_dtypes.bfloat16
    x = np.asarray(x, np.float32)
    W_qkv = np.asarray(W_qkv, np.float32)
    W_proj = np.asarray(W_proj, np.float32)
    rel_pos_t = np.asarray(rel_pos_t, np.float32)

    sel = np.zeros((NROWS, TLOC), np.float32)
    sel[np.arange(NROWS), np.arange(NROWS) // S] = 1.0 / S
    sel = np.ascontiguousarray(sel.astype(bf))
    rel_eff = rel_pos_t / SCALE                            # (127, HD)
    wq_b = np.ascontiguousarray(W_qkv[:, :D].astype(bf))
    wkv_b = np.ascontiguousarray(W_qkv[:, D:].astype(bf))
    wp_b = np.ascontiguousarray(W_proj.astype(bf))
    gq_b = np.ascontiguousarray(np.broadcast_to(np.tile(np.asarray(g_q, np.float32), NH), (TLOC, D)))
    bq_b = np.ascontiguousarray(np.broadcast_to(np.tile(np.asarray(b_q, np.float32), NH), (TLOC, D)))
    gkv_row = np.concatenate([np.tile(np.asarray(g_k, np.float32), NH),
                              np.tile(np.asarray(g_v, np.float32), NH)])
    bkv_row = np.concatenate([np.tile(np.asarray(b_k, np.float32), NH),
                              np.tile(np.asarray(b_v, np.float32), NH)])
    gkv_b = np.ascontiguousarray(np.broadcast_to(gkv_row, (T, 2 * D)))
    bkv_b = np.ascontiguousarray(np.broadcast_to(bkv_row, (T, 2 * D)))
    bproj_b = np.ascontiguousarray(np.broadcast_to(np.asarray(b_proj, np.float32), (TLOC, D)))

    in_maps = []
    jj = np.arange(128)
    for c in range(N_CORES):
        b = c // 2
        t0 = (c % 2) * TLOC
        # R flipped per core: R_c[d, j] = rel_eff[clip(t0 + 126 - j), d]
        idx = np.clip(t0 + 126 - jj, 0, NDIST - 1)
        rptt_c = np.ascontiguousarray(rel_eff[idx].T.astype(bf))   # (HD, 128)
        in_maps.append({
            "xloc": np.ascontiguousarray(
                x[b, t0:t0 + TLOC].reshape(NROWS, D).astype(bf)),
            "sel": sel,
            "wq": wq_b,
            "wkv": wkv_b,
            "wp": wp_b,
            "rptt": rptt_c,
            "gq": gq_b, "bq": bq_b,
            "gkv": gkv_b, "bkv": bkv_b,
            "bproj": bproj_b,
        })
    return in_maps


def _get_nc():
    if "nc" not in _BUILD_CACHE:
        _BUILD_CACHE["nc"] = _build_nc()
    return _BUILD_CACHE["nc"]


def run_on_device(in_maps, **kw):
    from concourse.bass_utils import run_bass_kernel_spmd
    nc = _get_nc()
    return run_bass_kernel_spmd(nc, in_maps, list(range(N_CORES)), **kw)


def kernel(**inputs):
    in_maps = _host_prep(**inputs)
    res = run_on_device(in_maps)
    out = np.zeros((B, T, D), np.float32)
    for c in range(N_CORES):
        b = c // 2
        t0 = (c % 2) * TLOC
        out[b, t0:t0 + TLOC] = res.results[c]["out"]
    return out


# revision 14
# speedup vs baseline: 1.5549x; 1.1024x over previous
"""AdaptivePoolAttention Trainium2 kernel (8 NeuronCores, SPMD).

Key algebraic restructure: AdaptiveAvgPool3d over spatial (H,W) commutes with
the qkv linear projection, so we pool x first (mean over H*W) and run the qkv
GEMM on the pooled (B,T,D) tensor. This turns the dominant-GEMM problem into a
memory-bound spatial reduction plus a small attention tail.

Sharding: core c handles batch b=c//2, token half c%2 (32 of 64 tokens).
 - Phase 1: each core pools its 32 tokens (spatial mean via a bf16 selector
   matmul on the TensorEngine, accumulated in PSUM).
 - Pairwise AllGather (cores 2c, 2c+1) of the pooled halves -> full (64, 768).
 - Phase 2b (overlaps the collective): q projection + LN for the local 32
   query tokens, plus the temporal rel-pos bias. The bias
   bias[t,s,h] = sum_d q[t,h,d] * rel_pos_t[t-s+63, d] is computed as one
   GEMM G = q @ R^T over the 127 distinct distances, then gathered into
   (t, h, s) layout with a single affine-strided DMA through DRAM
   (address = t*1535 + h*128 + s + 63 after a per-core column flip of R).
 - Phase 3: k/v projection + LN for all 64 tokens, per-head attention,
   softmax (no max-subtraction: |logits| < 5), A@V, residual, out projection.
Host side shards/preps inputs (incl. fp32->bf16 for the big operands) and
concatenates the 8 disjoint (32, 768) output row-blocks.
"""

import numpy as np
from contextlib import ExitStack

B, T, NH, HD, D = 4, 64, 12, 64, 768
S = 196            # 14*14 spatial positions
TLOC = 32          # tokens per core
NROWS = TLOC * S   # 6272 rows of x per core
NT = 49            # 128-row tiles per core
G = 7              # tiles per DMA group
KB = D // 128      # 6 contraction tiles of 128
NDIST = 2 * T - 1  # 127 distinct temporal distances
SCALE = HD ** -0.5
LN_EPS = 1e-5
N_CORES = 8

_BUILD_CACHE = {}


def _build_nc():
    import concourse.bass as bass
    import concourse.bacc as bacc
    import concourse.tile as tile
    import concourse.mybir as mybir
    from concourse.masks import make_identity
    from concourse.tile_rust import add_dep_helper

    f32 = mybir.dt.float32
    bf16 = mybir.dt.bfloat16

    nc = bacc.Bacc(
        "TRN2", target_bir_lowering=False, debug=False, num_devices=N_CORES,
    )

    xloc = nc.declare_dram_parameter("xloc", [NROWS, D], bf16, isOutput=False)
    sel = nc.declare_dram_parameter("sel", [128, NT, TLOC], bf16, isOutput=False)
    wq = nc.declare_dram_parameter("wq", [D, D], bf16, isOutput=False)
    wkv = nc.declare_dram_parameter("wkv", [D, 2 * D], bf16, isOutput=False)
    wp = nc.declare_dram_parameter("wp", [D, D], bf16, isOutput=False)
    rptt = nc.declare_dram_parameter("rptt", [HD, 128], bf16, isOutput=False)
    gq = nc.declare_dram_parameter("gq", [TLOC, D], f32, isOutput=False)
    bq = nc.declare_dram_parameter("bq", [TLOC, D], f32, isOutput=False)
    gkv = nc.declare_dram_parameter("gkv", [T, 2 * D], f32, isOutput=False)
    bkv = nc.declare_dram_parameter("bkv", [T, 2 * D], f32, isOutput=False)
    bproj = nc.declare_dram_parameter("bproj", [TLOC, D], f32, isOutput=False)
    out_ext = nc.declare_dram_parameter("out", [TLOC, D], f32, isOutput=True)

    with ExitStack() as ctx:
        tc = ctx.enter_context(tile.TileContext(nc))
        const = ctx.enter_context(tc.tile_pool(name="const", bufs=1))
        xp = ctx.enter_context(tc.tile_pool(name="xp", bufs=2))
        sb = ctx.enter_context(tc.tile_pool(name="sb", bufs=1))
        pg = ctx.enter_context(tc.tile_pool(name="pg", bufs=2, space="PSUM"))
        pt = ctx.enter_context(tc.tile_pool(name="pt", bufs=2, space="PSUM"))
        dram = ctx.enter_context(tc.tile_pool(name="dram", bufs=1, space="DRAM"))

        ident = const.tile([128, 128], bf16, tag="ident")
        make_identity(nc, ident)
        eps_sb = const.tile([128, 1], f32, tag="eps")
        nc.vector.memset(eps_sb, LN_EPS)
        zero_sb = const.tile([128, 1], f32, tag="zero")
        nc.vector.memset(zero_sb, 0.0)

        # ---- phase 1 inputs first on the sync ring: sel, then x groups ----
        sel_sb = const.tile([128, NT, TLOC], bf16, tag="sel")
        nc.sync.dma_start(out=sel_sb, in_=sel.ap())

        x_r = xloc.ap().rearrange("(g i p) d -> g p i d", i=G, p=128)
        m_psum = pg.tile([TLOC, D], f32, tag="g")
        x_dmas = []
        for g in range(G):
            xt = xp.tile([128, G, D], bf16, tag="x")
            ring = nc.sync if g % 2 == 0 else nc.scalar
            x_dmas.append(ring.dma_start(out=xt, in_=x_r[g]))
            for i in range(G):
                ti = g * G + i
                for c0, cw in ((0, 512), (512, 256)):
                    nc.tensor.matmul(
                        m_psum[:, c0:c0 + cw],
                        sel_sb[:, ti, :],
                        xt[:, i, c0:c0 + cw],
                        start=(ti == 0),
                        stop=(ti == NT - 1),
                    )
        m_sb = sb.tile([TLOC, D], bf16, tag="m")
        nc.vector.tensor_copy(out=m_sb, in_=m_psum)

        # ---- weights ordered after the x stream, split across both rings ----
        last_sync = x_dmas[-1] if (G - 1) % 2 == 0 else x_dmas[-2]
        last_scal = x_dmas[-1] if (G - 1) % 2 == 1 else x_dmas[-2]
        wq_sb = const.tile([128, KB, D], bf16, tag="wq")
        wd = nc.sync.dma_start(out=wq_sb, in_=wq.ap().rearrange("(k p) e -> p k e", p=128))
        add_dep_helper(wd.ins, last_sync.ins, sync=False, reason="x before weights")
        wkv_sb = const.tile([128, KB, 2 * D], bf16, tag="wkv")
        wd = nc.scalar.dma_start(out=wkv_sb, in_=wkv.ap().rearrange("(k p) e -> p k e", p=128))
        add_dep_helper(wd.ins, last_scal.ins, sync=False, reason="x before weights")
        wp_sb = const.tile([128, KB, D], bf16, tag="wp")
        wd = nc.sync.dma_start(out=wp_sb, in_=wp.ap().rearrange("(k p) e -> p k e", p=128))
        add_dep_helper(wd.ins, last_sync.ins, sync=False, reason="x before weights")

        # small constants on the scalar ring; gpsimd stays free for the collective
        rptt_sb = const.tile([HD, 128], bf16, tag="rptt")
        nc.scalar.dma_start(out=rptt_sb, in_=rptt.ap())
        gq_sb = const.tile([TLOC, D], f32, tag="gq")
        nc.scalar.dma_start(out=gq_sb, in_=gq.ap())
        bq_sb = const.tile([TLOC, D], f32, tag="bq")
        nc.scalar.dma_start(out=bq_sb, in_=bq.ap())
        gkv_sb = const.tile([T, 2 * D], f32, tag="gkv")
        nc.scalar.dma_start(out=gkv_sb, in_=gkv.ap())
        bkv_sb = const.tile([T, 2 * D], f32, tag="bkv")
        nc.scalar.dma_start(out=bkv_sb, in_=bkv.ap())
        bproj_sb = const.tile([TLOC, D], f32, tag="bproj")
        nc.scalar.dma_start(out=bproj_sb, in_=bproj.ap())

        # ---- pairwise AllGather of pooled halves (bf16) ----
        ag_in = dram.tile([TLOC, D], bf16, tag="agi")
        ag_out = dram.tile([T, D], bf16, tag="ago")
        nc.gpsimd.dma_start(out=ag_in, in_=m_sb)
        nc.gpsimd.collective_compute(
            "AllGather",
            mybir.AluOpType.bypass,
            replica_groups=[[0, 1], [2, 3], [4, 5], [6, 7]],
            ins=[ag_in.opt()],
            outs=[ag_out.opt()],
        )
        mf_sb = sb.tile([T, D], bf16, tag="mf")
        nc.sync.dma_start(out=mf_sb, in_=ag_out)

        def bcast_free(ap2d, inner):
            # (P, F) AP -> (P, F, inner) AP with stride-0 innermost broadcast
            return bass.AP(
                tensor=ap2d.tensor,
                offset=ap2d.offset,
                ap=[*ap2d.ap, [0, inner]],
            )

        def layer_norm(src_psum, n_part, n_groups, g_tile, b_tile, out_tile, nm):
            # src (n_part, n_groups*64): per-64-group LN, batched DVE ops
            src3 = src_psum.rearrange("p (g d) -> p g d", g=n_groups)
            mean = sb.tile([n_part, n_groups], f32, tag=f"{nm}_mean")
            nc.vector.reduce_sum(out=mean, in_=src3, axis=mybir.AxisListType.X)
            nc.vector.tensor_scalar_mul(out=mean, in0=mean, scalar1=1.0 / HD)
            xc = sb.tile([n_part, n_groups, HD], f32, tag=f"{nm}_xc")
            nc.vector.tensor_tensor(
                out=xc, in0=src3, in1=bcast_free(mean[:], HD),
                op=mybir.AluOpType.subtract,
            )
            sq = sb.tile([n_part, n_groups, HD], f32, tag=f"{nm}_sq")
            nc.vector.tensor_mul(out=sq, in0=xc, in1=xc)
            var = sb.tile([n_part, n_groups], f32, tag=f"{nm}_var")
            nc.vector.reduce_sum(out=var, in_=sq, axis=mybir.AxisListType.X)
            # std = sqrt(var/HD + eps); rstd = 1/std
            nc.scalar.activation(
                out=var, in_=var, func=mybir.ActivationFunctionType.Sqrt,
                bias=eps_sb[:n_part], scale=1.0 / HD,
            )
            nc.vector.reciprocal(out=var, in_=var)
            nc.vector.tensor_tensor(
                out=xc, in0=xc, in1=bcast_free(var[:], HD),
                op=mybir.AluOpType.mult,
            )
            xcf = xc.rearrange("p g d -> p (g d)")
            nc.vector.tensor_mul(out=xcf, in0=xcf, in1=g_tile)
            nc.vector.tensor_add(out=out_tile, in0=xcf, in1=b_tile)

        # ---- phase 2b: q path (local tokens; overlaps the collective) ----
        mT_psum = pt.tile([128, KB, TLOC], bf16, tag="t")
        for k in range(KB):
            nc.tensor.matmul(
                mT_psum[:, k, :], m_sb[:, k * 128:(k + 1) * 128],
                ident[:TLOC, :TLOC], is_transpose=True,
            )
        mT_sb = sb.tile([128, KB, TLOC], bf16, tag="mT")
        nc.any.tensor_copy(out=mT_sb, in_=mT_psum)

        q_psum = pg.tile([TLOC, D], f32, tag="g")
        for k in range(KB):
            for c0, cw in ((0, 512), (512, 256)):
                nc.tensor.matmul(
                    q_psum[:, c0:c0 + cw],
                    mT_sb[:, k, :],
                    wq_sb[:, k, c0:c0 + cw],
                    start=(k == 0), stop=(k == KB - 1),
                )
        ln_q = sb.tile([TLOC, D], bf16, tag="lnq")
        layer_norm(q_psum, TLOC, NH, gq_sb, bq_sb, ln_q, "q")

        # q^T in per-head layout (64 d, NH heads, 32 t)
        qbT_psum = pt.tile([HD, NH, TLOC], bf16, tag="t")
        for h in range(NH):
            nc.tensor.matmul(
                qbT_psum[:, h, :], ln_q[:, h * HD:(h + 1) * HD],
                ident[:TLOC, :TLOC], is_transpose=True,
            )
        qbT_sb = sb.tile([HD, NH, TLOC], bf16, tag="qbT")
        nc.any.tensor_copy(out=qbT_sb, in_=qbT_psum)

        # rel-pos bias: G[t, h, j] = sum_d q[t,h,d] * Rflip[d, j], then an
        # affine gather through DRAM turns G into bias[t, h, s] (j = 63-t+s).
        g_psum = pg.tile([TLOC, NH, 128], f32, tag="g")
        for h in range(NH):
            nc.tensor.matmul(
                g_psum[:, h, :], qbT_sb[:, h, :], rptt_sb,
                start=True, stop=True,
            )
        g_sb = sb.tile([TLOC, NH, 128], f32, tag="gsb")
        nc.any.tensor_copy(out=g_sb, in_=g_psum)
        g_dram = dram.tile([TLOC, NH, 128], f32, tag="gd")
        nc.sync.dma_start(out=g_dram, in_=g_sb)
        bias_sb = sb.tile([TLOC, NH, T], f32, tag="bias")
        gather_ap = bass.AP(
            tensor=g_dram.tensor,
            offset=g_dram.offset + 63,
            ap=[[NH * 128 - 1, TLOC], [128, NH], [1, T]],
        )
        nc.sync.dma_start(out=bias_sb, in_=gather_ap)

        # ---- phase 3: kv path on gathered tokens ----
        mfT_psum = pt.tile([128, KB, T], bf16, tag="t")
        for k in range(KB):
            nc.tensor.matmul(
                mfT_psum[:, k, :], mf_sb[:, k * 128:(k + 1) * 128],
                ident[:T, :T], is_transpose=True,
            )
        mfT_sb = sb.tile([128, KB, T], bf16, tag="mfT")
        nc.any.tensor_copy(out=mfT_sb, in_=mfT_psum)

        k_psum = pg.tile([T, D], f32, tag="g")
        v_psum = pg.tile([T, D], f32, tag="g")
        for k in range(KB):
            for c0, cw in ((0, 512), (512, 256)):
                nc.tensor.matmul(
                    k_psum[:, c0:c0 + cw],
                    mfT_sb[:, k, :],
                    wkv_sb[:, k, c0:c0 + cw],
                    start=(k == 0), stop=(k == KB - 1),
                )
            for c0, cw in ((0, 512), (512, 256)):
                nc.tensor.matmul(
                    v_psum[:, c0:c0 + cw],
                    mfT_sb[:, k, :],
                    wkv_sb[:, k, D + c0:D + c0 + cw],
                    start=(k == 0), stop=(k == KB - 1),
                )
        ln_k = sb.tile([T, D], bf16, tag="lnk")
        layer_norm(k_psum, T, NH, gkv_sb[:, :D], bkv_sb[:, :D], ln_k, "k")
        ln_v = sb.tile([T, D], bf16, tag="lnv")
        layer_norm(v_psum, T, NH, gkv_sb[:, D:], bkv_sb[:, D:], ln_v, "v")

        # k^T per head: (64 d, NH, 64 s)
        kT_psum = pt.tile([HD, NH, T], bf16, tag="t")
        for h in range(NH):
            nc.tensor.matmul(
                kT_psum[:, h, :], ln_k[:, h * HD:(h + 1) * HD],
                ident[:T, :T], is_transpose=True,
            )
        kT_sb = sb.tile([HD, NH, T], bf16, tag="kT")
        nc.any.tensor_copy(out=kT_sb, in_=kT_psum)

        # scores = q @ k^T, then add bias, then exp (scaled)
        s_psum = pg.tile([TLOC, NH, T], f32, tag="g")
        for h in range(NH):
            nc.tensor.matmul(
                s_psum[:, h, :], qbT_sb[:, h, :], kT_sb[:, h, :],
                start=True, stop=True,
            )
        s_sb = sb.tile([TLOC, NH, T], f32, tag="ssb")
        nc.vector.tensor_add(out=s_sb, in0=s_psum, in1=bias_sb)
        p_sb = sb.tile([TLOC, NH, T], bf16, tag="p")
        nc.scalar.activation(
            out=p_sb.rearrange("p h s -> p (h s)"),
            in_=s_sb.rearrange("p h s -> p (h s)"),
            func=mybir.ActivationFunctionType.Exp,
            bias=zero_sb[:TLOC], scale=SCALE,
        )
        rsum = sb.tile([TLOC, NH], f32, tag="rsum")
        nc.vector.reduce_sum(out=rsum, in_=p_sb, axis=mybir.AxisListType.X)
        nc.vector.reciprocal(out=rsum, in_=rsum)

        # P^T per head
        pT_psum = pt.tile([T, NH, TLOC], bf16, tag="t")
        for h in range(NH):
            nc.tensor.matmul(
                pT_psum[:, h, :], p_sb[:, h, :],
                ident[:TLOC, :TLOC], is_transpose=True,
            )
        pT_sb = sb.tile([T, NH, TLOC], bf16, tag="pT")
        nc.any.tensor_copy(out=pT_sb, in_=pT_psum)

        # A@V per head
        o_psum = pg.tile([TLOC, NH, HD], f32, tag="g")
        for h in range(NH):
            nc.tensor.matmul(
                o_psum[:, h, :], pT_sb[:, h, :],
                ln_v[:, h * HD:(h + 1) * HD],
                start=True, stop=True,
            )
        # o = o * (1/sum) + ln_q (residual)
        o_nrm = sb.tile([TLOC, NH, HD], bf16, tag="onrm")
        for h in range(NH):
            nc.vector.tensor_scalar_mul(
                out=o_nrm[:, h, :], in0=o_psum[:, h, :], scalar1=rsum[:, h:h + 1],
            )
        o_sb = sb.tile([TLOC, D], bf16, tag="o")
        nc.vector.tensor_add(
            out=o_sb, in0=o_nrm.rearrange("p h d -> p (h d)"), in1=ln_q,
        )

        # o^T then output projection
        oT_psum = pt.tile([128, KB, TLOC], bf16, tag="t")
        for k in range(KB):
            nc.tensor.matmul(
                oT_psum[:, k, :], o_sb[:, k * 128:(k + 1) * 128],
                ident[:TLOC, :TLOC], is_transpose=True,
            )
        oT_sb = sb.tile([128, KB, TLOC], bf16, tag="oT")
        nc.any.tensor_copy(out=oT_sb, in_=oT_psum)

        proj_psum = pg.tile([TLOC, D], f32, tag="g")
        for k in range(KB):
            for c0, cw in ((0, 512), (512, 256)):
                nc.tensor.matmul(
                    proj_psum[:, c0:c0 + cw],
                    oT_sb[:, k, :],
                    wp_sb[:, k, c0:c0 + cw],
                    start=(k == 0), stop=(k == KB - 1),
                )
        out_sb = sb.tile([TLOC, D], f32, tag="outsb")
        nc.vector.tensor_add(out=out_sb, in0=proj_psum, in1=bproj_sb)
        nc.gpsimd.dma_start(out=out_ext.ap(), in_=out_sb)

    nc.compile()
    return nc


def _host_prep(x, W_qkv, g_q, b_q, g_k, b_k, g_v, b_v, W_proj, b_proj, rel_pos_t):
    import ml_dtypes
    bf = ml_dtypes.bfloat16
    x = np.asarray(x, np.float32)
    W_qkv = np.asarray(W_qkv, np.float32)
    W_proj = np.asarray(W_proj, np.float32)
    rel_pos_t = np.asarray(rel_pos_t, np.float32)

    sel = np.zeros((NROWS, TLOC), np.float32)
    sel[np.arange(NROWS), np.arange(NROWS) // S] = 1.0 / S
    # pre-layout to the SBUF tile shape (128 partitions, NT, TLOC)
    sel = np.ascontiguousarray(
        sel.reshape(NT, 128, TLOC).transpose(1, 0, 2).astype(bf))
    rel_eff = rel_pos_t / SCALE                            # (127, HD)
    wq_b = np.ascontiguousarray(W_qkv[:, :D].astype(bf))
    wkv_b = np.ascontiguousarray(W_qkv[:, D:].astype(bf))
    wp_b = np.ascontiguousarray(W_proj.astype(bf))
    gq_b = np.ascontiguousarray(np.broadcast_to(np.tile(np.asarray(g_q, np.float32), NH), (TLOC, D)))
    bq_b = np.ascontiguousarray(np.broadcast_to(np.tile(np.asarray(b_q, np.float32), NH), (TLOC, D)))
    gkv_row = np.concatenate([np.tile(np.asarray(g_k, np.float32), NH),
                              np.tile(np.asarray(g_v, np.float32), NH)])
    bkv_row = np.concatenate([np.tile(np.asarray(b_k, np.float32), NH),
                              np.tile(np.asarray(b_v, np.float32), NH)])
    gkv_b = np.ascontiguousarray(np.broadcast_to(gkv_row, (T, 2 * D)))
    bkv_b = np.ascontiguousarray(np.broadcast_to(bkv_row, (T, 2 * D)))
    bproj_b = np.ascontiguousarray(np.broadcast_to(np.asarray(b_proj, np.float32), (TLOC, D)))

    in_maps = []
    jj = np.arange(128)
    for c in range(N_CORES):
        b = c // 2
        t0 = (c % 2) * TLOC
        # R flipped per core: R_c[d, j] = rel_eff[clip(t0 + 126 - j), d]
        idx = np.clip(t0 + 126 - jj, 0, NDIST - 1)
        rptt_c = np.ascontiguousarray(rel_eff[idx].T.astype(bf))   # (HD, 128)
        in_maps.append({
            "xloc": np.ascontiguousarray(
                x[b, t0:t0 + TLOC].reshape(NROWS, D).astype(bf)),
            "sel": sel,
            "wq": wq_b,
            "wkv": wkv_b,
            "wp": wp_b,
            "rptt": rptt_c,
            "gq": gq_b, "bq": bq_b,
            "gkv": gkv_b, "bkv": bkv_b,
            "bproj": bproj_b,
        })
    return in_maps


def _get_nc():
    if "nc" not in _BUILD_CACHE:
        _BUILD_CACHE["nc"] = _build_nc()
    return _BUILD_CACHE["nc"]


def run_on_device(in_maps, **kw):
    from concourse.bass_utils import run_bass_kernel_spmd
    nc = _get_nc()
    return run_bass_kernel_spmd(nc, in_maps, list(range(N_CORES)), **kw)


def kernel(**inputs):
    in_maps = _host_prep(**inputs)
    res = run_on_device(in_maps)
    out = np.zeros((B, T, D), np.float32)
    for c in range(N_CORES):
        b = c // 2
        t0 = (c % 2) * TLOC
        out[b, t0:t0 + TLOC] = res.results[c]["out"]
    return out
